# revision 1
# baseline (speedup 1.0000x reference)
"""DKVMN forward Trainium2 Bass kernel (fp16 bulk path).

Model (per sample): embeddings -> softmax attention w over M memory slots ->
sequential memory update Mv_t = Mv_{t-1} * (1 - w_t e_t^T) + w_t a_t^T ->
weighted read of PRE-update memory -> output MLP -> sigmoid.

Sharding: data-parallel over batch. B=64 across 8 cores -> 8 samples/core.
Tables + weights replicated. Each core returns [8, 199]; host concatenates.

Key structure (per core, per sample; bulk tensors fp16, accum f32 on PE/PSUM):
- dma_gather k/v rows (fp16 tables) -> PE transpose -> kT/vT [D, L]
- e_T = sigmoid(eW.T @ vT), a_neg = -tanh(aW.T @ vT + a_b)  (ACT, fp16 out)
- w = softmax_m(k @ Mk.T) in [t, m] (f32 PSUM softmax) -> fp16 -> transpose
  -> w_flat [1, M*L] (m-major) -> PE K=1 broadcast mms -> PSUM -> ACT copy
  -> Wbc [128, M*L] fp16
- sign-trick scan (state_t = (-1)^t Mv_t):  data0 = We - 1,
  data1 = Wbc * a_mod  (a_mod = a * (-1)^{t+1}), op0=mult, op1=subtract.
  m-blocks chained in ONE scan instr per m-group via reset columns
  (data0=0, data1=+Mv0 -> state resets to -Mv0).
- read: G = Y_shift * Wbc written (t,m)-contiguous, in-place cumsum scan,
  boundary diff, sign fix; read_0 = sum_m w[0,m] Mv0.
- f = tanh(fWr.T @ read + fWk.T @ kT + f_b); p = sigmoid(pW @ f + p_b).
"""
import sys

sys.path.insert(0, "/opt/trn_rl_repo")

import os as _os

import numpy as np

import concourse.bacc as bacc
import concourse.bass as bass
import concourse.tile as tile
from concourse import library_config, mybir
from concourse.bass_utils import run_bass_kernel_spmd

f32 = mybir.dt.float32
f16 = mybir.dt.float16
i16 = mybir.dt.int16
AF = mybir.ActivationFunctionType
ALU = mybir.AluOpType
AX = mybir.AxisListType

B, L, NS, D, M = 64, 200, 1000, 128, 50
NCORES = 8
BL = B // NCORES          # samples per core
NIDX = 256                # padded gather idxs per sample (L=200 real)
MGRP = 10                 # m's per scan group
NGRP = M // MGRP          # 5 groups
GCOLS = MGRP * L          # 2000 w-cols per group
SCOLS = MGRP * (L + 1)    # 2010 scan cols (incl. reset col per m)
WCOLS = M * L             # 10000
CCOLS = M * L              # 10000: G cols, m-major [m, t]

TRACE = False
LAST_RESULTS = None

# engine assignment knobs (tuned against TimelineSim)
POOL_SCANS = int(_os.environ.get("K_POOL_SCANS", "0"))   # of NGRP scans (HW: Pool cannot scan)
ACT_SUBS = int(_os.environ.get("K_ACT_SUBS", "5"))       # of NGRP alpha-subs
POOL_BNS = int(_os.environ.get("K_POOL_BNS", "0"))       # of NGRP BN-mults
POOL_GS = int(_os.environ.get("K_POOL_GS", "4"))         # of NGRP G-mults


def _ap(t_ap, offset_add, free_dims):
    """Raw AP view: keep partition dim, replace free dims."""
    return bass.AP(t_ap.tensor, t_ap.offset + offset_add,
                   [t_ap.ap[0]] + free_dims)


def build_bass(n_samples=BL):
    BLn = n_samples
    nc = bacc.Bacc("TRN2", target_bir_lowering=False, debug=False,
                   num_devices=NCORES)

    def dram_in(name, shape, dtype=f32):
        return nc.dram_tensor(name, shape, dtype, kind="ExternalInput")

    k_emb = dram_in("k_emb", [NS, D], f16)
    v_emb = dram_in("v_emb", [2 * NS, D], f16)
    kidx = dram_in("kidx", [128, BLn * NIDX // 16], i16)
    vidx = dram_in("vidx", [128, BLn * NIDX // 16], i16)
    MkT = dram_in("MkT", [D, M], f16)
    eWT = dram_in("eWT", [D, D], f16)
    aWT = dram_in("aWT", [D, D], f16)
    fWrT = dram_in("fWrT", [D, D], f16)
    fWkT = dram_in("fWkT", [D, D], f16)
    pWT = dram_in("pWT", [D, 1], f16)
    Mv0T16 = dram_in("Mv0T16", [D, M], f16)
    ident = dram_in("ident", [D, D], f16)
    ones16 = dram_in("ones16", [1, D], f16)
    calt = dram_in("calt", [D, L], f16)     # (-1)^t : +1, -1, +1, ...
    csgn = dram_in("csgn", [D, L], f16)     # [+1, +1, -1, +1, -1, ...]
    e_b = dram_in("e_b", [D, 1])
    nab = dram_in("nab", [D, 1])            # -a_b
    f_b = dram_in("f_b", [D, 1])
    p_b1 = dram_in("p_b1", [1, 1])
    p_out = nc.dram_tensor("p_out", [BLn, L - 1], f32, kind="ExternalOutput")

    with tile.TileContext(nc) as tc:
        nc.gpsimd.load_library(library_config.mlp)
        with tc.tile_pool(name="const", bufs=1) as cpool, \
             tc.tile_pool(name="rows", bufs=2) as rpool, \
             tc.tile_pool(name="sm", bufs=2) as sm, \
             tc.tile_pool(name="wfp", bufs=2) as wfp, \
             tc.tile_pool(name="wbcp", bufs=2) as wbcp, \
             tc.tile_pool(name="unit", bufs=2) as unit, \
             tc.tile_pool(name="cc", bufs=2) as ccp, \
             tc.tile_pool(name="psA", bufs=1, space="PSUM") as psA, \
             tc.tile_pool(name="psW", bufs=1, space="PSUM") as psW, \
             tc.tile_pool(name="psB", bufs=1, space="PSUM") as psB, \
             tc.tile_pool(name="psBC", bufs=2, space="PSUM") as psBC:

            def cload(dram, shape, dtype=f32):
                t = cpool.tile(shape, dtype, tag=dram.name)
                nc.sync.dma_start(t[:], dram[(slice(None),) * len(shape)])
                return t

            c_MkT = cload(MkT, [D, M], f16)
            c_eWT = cload(eWT, [D, D], f16)
            c_aWT = cload(aWT, [D, D], f16)
            c_fWrT = cload(fWrT, [D, D], f16)
            c_fWkT = cload(fWkT, [D, D], f16)
            c_pWT = cload(pWT, [D, 1], f16)
            c_Mv0 = cload(Mv0T16, [D, M], f16)
            c_id = cload(ident, [D, D], f16)
            c_ones = cload(ones16, [1, D], f16)
            c_alt = cload(calt, [D, L], f16)
            c_sgn = cload(csgn, [D, L], f16)
            c_eb = cload(e_b, [D, 1])
            c_nab = cload(nab, [D, 1])
            c_fb = cload(f_b, [D, 1])
            c_pb = cload(p_b1, [1, 1])
            c_kidx = cload(kidx, [128, BLn * NIDX // 16], i16)
            c_vidx = cload(vidx, [128, BLn * NIDX // 16], i16)

            p_row = sm.tile([1, BLn * L], f32, tag="p_row")

            for b in range(BLn):
                isl = slice(b * (NIDX // 16), (b + 1) * (NIDX // 16))

                # ---- transposed gathers: directly [D, t] fp16 ----
                kT3 = rpool.tile([128, 1, NIDX], f16, tag="kT3")
                nc.gpsimd.dma_gather(kT3[:], k_emb[:, :], c_kidx[:, isl],
                                     num_idxs=NIDX, num_idxs_reg=L,
                                     elem_size=D, transpose=True)
                vT3 = rpool.tile([128, 1, NIDX], f16, tag="vT3")
                nc.gpsimd.dma_gather(vT3[:], v_emb[:, :], c_vidx[:, isl],
                                     num_idxs=NIDX, num_idxs_reg=L,
                                     elem_size=D, transpose=True)
                kT = _ap(kT3[:], 0, [[1, L]])
                vT = _ap(vT3[:], 0, [[1, L]])

                # ---- e / a_mod ----
                eps = psB.tile([D, L], f32, tag="ea")
                nc.tensor.matmul(eps[:], c_eWT[:], vT)
                e_T = sm.tile([D, L], f16, tag="e_T")
                nc.scalar.activation(e_T[:], eps[:], AF.Sigmoid,
                                     bias=c_eb[:], scale=1.0)
                aps = psB.tile([D, L], f32, tag="ea")
                nc.tensor.matmul(aps[:], c_aWT[:], vT)
                a_negT = sm.tile([D, L], f16, tag="a_negT")
                nc.scalar.activation(a_negT[:], aps[:], AF.Tanh,
                                     bias=c_nab[:], scale=-1.0)
                # a_mod = a * (-1)^{t+1} = a_neg * (-1)^t
                a_mod = sm.tile([D, L], f16, tag="a_mod")
                nc.gpsimd.tensor_tensor(a_mod[:], a_negT[:], c_alt[:],
                                        ALU.mult)

                # ---- w softmax (f32 psum) -> fp16 [m, t] ----
                wmT = sm.tile([M, L], f16, tag="wmT")
                for tb in range(2):
                    t0 = tb * 128
                    tsz = min(128, L - t0)
                    wps = psW.tile([128, M], f32, tag="wps")
                    nc.tensor.matmul(wps[0:tsz, :],
                                     _ap(kT3[:], t0, [[1, tsz]]),
                                     c_MkT[:])
                    negmax = sm.tile([128, 1], f32, tag="negmax")
                    nc.vector.tensor_reduce(negmax[0:tsz, :], wps[0:tsz, :],
                                            AX.X, ALU.max, negate=True)
                    wexp = sm.tile([128, M], f32, tag="wexp")
                    nc.scalar.activation(wexp[0:tsz, :], wps[0:tsz, :],
                                         AF.Exp, bias=negmax[0:tsz, :],
                                         scale=1.0)
                    ssum = sm.tile([128, 1], f32, tag="ssum")
                    nc.vector.tensor_reduce(ssum[0:tsz, :], wexp[0:tsz, :],
                                            AX.X, ALU.add)
                    rcp = sm.tile([128, 1], f32, tag="rcp")
                    nc.vector.reciprocal(rcp[0:tsz, :], ssum[0:tsz, :])
                    w16 = sm.tile([128, M], f16, tag="w16")
                    nc.vector.tensor_scalar_mul(w16[0:tsz, :], wexp[0:tsz, :],
                                                rcp[0:tsz, :])
                    wtp = psW.tile([M, 128], f16, tag="wps")
                    nc.tensor.transpose(wtp[:, 0:tsz], w16[0:tsz, :],
                                        c_id[0:tsz, 0:tsz])
                    nc.vector.tensor_copy(wmT[:, t0:t0 + tsz],
                                          wtp[:, 0:tsz])

                # ---- w_flat [1, M*L] fp16 (m-major) -> Wbc via PE + ACT ----
                w_flat = wfp.tile([1, WCOLS], f16, tag="w_flat")
                nc.sync.dma_start(
                    _ap(w_flat[:], 0, [[L, M], [1, L]]), wmT[:])
                Wbc = wbcp.tile([128, WCOLS], f16, tag="Wbc")
                NBC = 10          # psum chunks per sample
                BCW = WCOLS // NBC       # 1000 cols per chunk
                for cch in range(NBC):
                    # [128, 1024] = 2 banks; 500-col matmuls at bank starts
                    bps = psBC.tile([128, 1024], f32, tag="bc")
                    for sub in range(2):
                        nc.tensor.matmul(
                            bps[:, sub * 512:sub * 512 + 500], c_ones[:],
                            w_flat[0:1, cch * BCW + sub * 500:
                                   cch * BCW + sub * 500 + 500])
                    nc.scalar.activation(
                        _ap(Wbc[:], cch * BCW, [[500, 2], [1, 500]]),
                        _ap(bps[:], 0, [[512, 2], [1, 500]]),
                        AF.Copy, bias=0.0, scale=1.0)

                # ---- per m-group: We-1, BN', chained scan, G ----
                C = ccp.tile([128, CCOLS], f16, tag="C")
                for g in range(NGRP):
                    g0 = g * GCOLS
                    m0 = g * MGRP

                    NW = unit.tile([128, SCOLS], f16, tag="NW")
                    nw_s = _ap(NW[:], 1, [[L + 1, MGRP], [1, L]])
                    wb_g = _ap(Wbc[:], g0, [[L, MGRP], [1, L]])
                    e_bc = _ap(e_T[:], 0, [[0, MGRP], [1, L]])
                    nc.vector.tensor_tensor(nw_s, wb_g, e_bc, ALU.mult)
                    # alpha' = We - 1
                    if g < ACT_SUBS:
                        nc.scalar.activation(nw_s, nw_s, AF.Copy,
                                             bias=-1.0, scale=1.0)
                    else:
                        nc.vector.tensor_scalar(nw_s, nw_s, 1.0, None,
                                                ALU.subtract)
                    # reset cols: data0 = 0
                    nc.vector.memset(_ap(NW[:], 0, [[L + 1, MGRP]]), 0.0)

                    BN = unit.tile([128, SCOLS], f16, tag="BN")
                    bn_s = _ap(BN[:], 1, [[L + 1, MGRP], [1, L]])
                    a_bc = _ap(a_mod[:], 0, [[0, MGRP], [1, L]])
                    bn_eng = nc.gpsimd if g < POOL_BNS else nc.vector
                    bn_eng.tensor_tensor(bn_s, wb_g, a_bc, ALU.mult)
                    # reset cols: data1 = +Mv0 (state <- -Mv0)
                    nc.vector.tensor_copy(_ap(BN[:], 0, [[L + 1, MGRP]]),
                                          c_Mv0[:, m0:m0 + MGRP])

                    Y = unit.tile([128, SCOLS], f16, tag="Y")
                    eng = nc.gpsimd if g < POOL_SCANS else nc.vector
                    eng.tensor_tensor_scan(Y[:], NW[:], BN[:], 0.0,
                                           ALU.mult, ALU.subtract)

                    # G into C, m-major: C[m*L + t] =
                    #   Y[m, t-1] * Wbc[m, t],  t = 1..199
                    c_v = _ap(C[:], m0 * L + 1, [[L, MGRP], [1, L - 1]])
                    y_v = _ap(Y[:], 1, [[L + 1, MGRP], [1, L - 1]])
                    w_v = _ap(Wbc[:], g0 + 1, [[L, MGRP], [1, L - 1]])
                    g_eng = nc.gpsimd if g < POOL_GS else nc.vector
                    g_eng.tensor_tensor(c_v, y_v, w_v, ALU.mult)

                # t=0 cols of C: w[0, m] * Mv0[:, m]
                t0w = _ap(Wbc[:], 0, [[L, M]])
                nc.vector.tensor_tensor(_ap(C[:], 0, [[L, M]]),
                                        c_Mv0[:], t0w, ALU.mult)

                # ---- f: m-reduction fused into PSUM accumulation ----
                # fps = sum_m fWr.T @ C_m + fWk.T @ (kT * csgn); C carries
                # (-1)^{t-1} for t>=1 so f_pre_true = fps * csgn
                kTs = sm.tile([D, L], f16, tag="kTs")
                nc.vector.tensor_tensor(kTs[:], kT, c_sgn[:], ALU.mult)
                fps = psB.tile([D, L], f32, tag="ea")
                for m in range(M):
                    nc.tensor.matmul(
                        fps[:], c_fWrT[:],
                        _ap(C[:], m * L, [[1, L]]),
                        start=(m == 0), stop=False, skip_group_check=True)
                nc.tensor.matmul(fps[:], c_fWkT[:], kTs[:],
                                 start=False, stop=True, skip_group_check=True)
                fcomb = sm.tile([D, L], f16, tag="fcomb")
                nc.vector.tensor_tensor(fcomb[:], fps[:], c_sgn[:], ALU.mult)
                f_T = sm.tile([D, L], f16, tag="f_T")
                nc.scalar.activation(f_T[:], fcomb[:], AF.Tanh,
                                     bias=c_fb[:], scale=1.0)
                pps = psB.tile([1, L], f32, tag="ea")
                nc.tensor.matmul(pps[:], c_pWT[:], f_T[:])
                nc.scalar.activation(p_row[0:1, b * L:(b + 1) * L], pps[:],
                                     AF.Sigmoid, bias=c_pb[:], scale=1.0)

            nc.sync.dma_start(p_out[:, :],
                              _ap(p_row[:], 1, [[L, BLn], [1, L - 1]]))

    nc.compile()
    return nc


def _idx_table(ids):
    """ids [n, L] -> dma_gather idx table [128, n*NIDX/16] int16."""
    out = np.empty((128, ids.shape[0] * NIDX // 16), np.int16)
    for b in range(ids.shape[0]):
        pad = np.full(NIDX, -1, np.int16)
        pad[:L] = ids[b]
        tab = np.tile(pad.reshape(NIDX // 16, 16).T, (8, 1))
        out[:, b * (NIDX // 16):(b + 1) * (NIDX // 16)] = tab
    return out


def _csgn():
    s = np.ones((D, L), np.float16)
    s[:, 2::2] = -1.0
    return s


def make_common(k_emb, v_emb, Mk, Mv0, e_W, e_b, a_W, a_b, f_W, f_b,
                p_W, p_b):
    alt = np.ones((D, L), np.float16)
    alt[:, 1::2] = -1.0
    return {
        "k_emb": np.asarray(k_emb, np.float16),
        "v_emb": np.asarray(v_emb, np.float16),
        "MkT": np.ascontiguousarray(np.asarray(Mk, np.float16).T),
        "eWT": np.ascontiguousarray(np.asarray(e_W, np.float16).T),
        "aWT": np.ascontiguousarray(np.asarray(a_W, np.float16).T),
        "fWrT": np.ascontiguousarray(np.asarray(f_W, np.float16)[:, :D].T),
        "fWkT": np.ascontiguousarray(np.asarray(f_W, np.float16)[:, D:].T),
        "pWT": np.ascontiguousarray(np.asarray(p_W, np.float16).T),
        "Mv0T16": np.ascontiguousarray(np.asarray(Mv0, np.float16).T),
        "ident": np.eye(D, dtype=np.float16),
        "ones16": np.ones((1, D), np.float16),
        "calt": alt,
        "csgn": _csgn(),
        "e_b": np.asarray(e_b, np.float32).reshape(D, 1),
        "nab": (-np.asarray(a_b, np.float32)).reshape(D, 1),
        "f_b": np.asarray(f_b, np.float32).reshape(D, 1),
        "p_b1": np.asarray(p_b, np.float32).reshape(1, 1),
    }


def kernel(skills, responses, k_emb, v_emb, Mk, Mv0,
           e_W, e_b, a_W, a_b, f_W, f_b, p_W, p_b):
    skills = np.asarray(skills)
    responses = np.asarray(responses)

    masked_r = responses * (responses > -1).astype(responses.dtype)
    x = (skills.astype(np.int64) + NS * masked_r.astype(np.int64))

    common = make_common(k_emb, v_emb, Mk, Mv0, e_W, e_b, a_W, a_b,
                         f_W, f_b, p_W, p_b)

    in_maps = []
    for c in range(NCORES):
        bsl = slice(c * BL, (c + 1) * BL)
        m = dict(common)
        m["kidx"] = _idx_table(skills[bsl])
        m["vidx"] = _idx_table(x[bsl])
        in_maps.append(m)

    nc = build_bass()
    global LAST_RESULTS
    res = run_bass_kernel_spmd(nc, in_maps, core_ids=list(range(NCORES)),
                               trace=TRACE)
    LAST_RESULTS = res
    out = np.concatenate([res.results[c]["p_out"] for c in range(NCORES)],
                         axis=0)
    return out.astype(np.float32)

